# revision 3
# baseline (speedup 1.0000x reference)
"""GQA + RoPE + causal attention + out-proj, sharded over 8 NeuronCores.

Sharding: core = 4*b + g  (b = batch 0..1, g = KV group 0..3).
Each core computes q/k/v projections for its (batch, group), RoPE, causal
attention for its 4 query heads, and the partial out-projection through its
256 rows of Wo. The host sums the 4 group-partials per batch (the all-reduce
of the row-sharded out projection) and stacks batches.

On-chip layout: everything token-on-free ("transposed"): xT [din, tok] built
via PE transposes, qT/kT [dh, tok], scores computed as scoresT [tk, tq] so
that softmax denominators come for free from a ones-row appended to the
(token-major) V tiles, and attnT feeds both the AV matmul and the
out-projection without any attention-sized transposes.
Softmax skips max-subtraction: scores * T**-0.5 have |x| < 1 for this
problem's scale (weights ~ 0.02 * randn), so exp never overflows.
"""

import os
import sys

for _p in ("/opt/trn_rl_repo",):
    if _p not in sys.path and os.path.isdir(_p):
        sys.path.insert(0, _p)

import ml_dtypes
import numpy as np

import concourse.bacc as bacc
import concourse.mybir as mybir
import concourse.tile as tile

F32 = mybir.dt.float32
BF16 = mybir.dt.bfloat16
EXP = mybir.ActivationFunctionType.Exp

B, T, DIN, DOUT = 2, 2048, 1024, 1024
G, H = 4, 16
HPG = H // G          # 4 query heads per group
DH = DOUT // H        # 64
QCOLS = HPG * DH      # 256 q columns per group
SCALE = float(T) ** -0.5
NCORES = 8

_CACHE = {}


def _build_nc():
    nc = bacc.Bacc("TRN2", target_bir_lowering=False, debug=False,
                   num_devices=NCORES)

    x_d = nc.dram_tensor("x", [T, DIN], F32, kind="ExternalInput")
    wq_d = nc.dram_tensor("wq", [DIN, QCOLS], F32, kind="ExternalInput")
    wkv_d = nc.dram_tensor("wkv", [DIN, 2 * DH], F32, kind="ExternalInput")
    wo_d = nc.dram_tensor("wo", [QCOLS, DOUT], F32, kind="ExternalInput")
    crep_d = nc.dram_tensor("crep", [128, T], BF16, kind="ExternalInput")
    srep_d = nc.dram_tensor("srep", [128, T], BF16, kind="ExternalInput")
    masks_d = nc.dram_tensor("masks", [128, 4, 512], BF16, kind="ExternalInput")
    idb_d = nc.dram_tensor("idb", [128, 128], BF16, kind="ExternalInput")
    out_d = nc.dram_tensor("out", [T, DOUT], F32, kind="ExternalOutput")

    with tile.TileContext(nc) as tc:
        _body(tc, nc, x_d, wq_d, wkv_d, wo_d, crep_d, srep_d, masks_d, idb_d,
              out_d)
    nc.compile()
    return nc


def _body(tc, nc, x_d, wq_d, wkv_d, wo_d, crep_d, srep_d, masks_d, idb_d,
          out_d):
    xap = x_d.ap()
    oap = out_d.ap()

    with (
        tc.tile_pool(name="cpool", bufs=1) as cpool,
        tc.tile_pool(name="bpool", bufs=1) as bpool,
        tc.tile_pool(name="wpool", bufs=1) as wpool,
        tc.tile_pool(name="ppool", bufs=1, space="PSUM") as ppool,
    ):
        # ---------------- constants / weights ----------------
        crep = cpool.tile([128, T], BF16, tag="crep")
        nc.sync.dma_start(crep, crep_d.ap())
        srep = cpool.tile([128, T], BF16, tag="srep")
        nc.sync.dma_start(srep, srep_d.ap())
        masks = cpool.tile([128, 4, 512], BF16, tag="masks")
        nc.sync.dma_start(masks, masks_d.ap())
        idb = cpool.tile([128, 128], BF16, tag="idb")
        nc.sync.dma_start(idb, idb_d.ap())

        wq = cpool.tile([128, 8, QCOLS], BF16, tag="wq")
        nc.gpsimd.dma_start(wq, wq_d.ap().rearrange("(c p) m -> p c m", p=128))
        wkv = cpool.tile([128, 8, 2 * DH], BF16, tag="wkv")
        nc.gpsimd.dma_start(wkv, wkv_d.ap().rearrange("(c p) m -> p c m", p=128))
        wo = cpool.tile([128, 2, DOUT], BF16, tag="wo")
        nc.gpsimd.dma_start(wo, wo_d.ap().rearrange("(r p) n -> p r n", p=128))

        # ---------------- persistent activations ----------------
        xt = bpool.tile([128, 8, T], BF16, tag="xt")       # xT, din chunk c
        qp0 = bpool.tile([128, T], BF16, tag="qp0")        # heads 0,1 (RoPEd)
        qp1 = bpool.tile([128, T], BF16, tag="qp1")        # heads 2,3
        k2 = bpool.tile([128, T], BF16, tag="k2")          # kT dup at base 0/64
        vst = bpool.tile([64, T], BF16, tag="vst")         # vT staging
        vex = bpool.tile([128, 16, DH + 1], BF16, tag="vex")  # [tok,65] + ones
        o0 = bpool.tile([128, T], BF16, tag="o0")          # o_gT heads 0,1
        o1 = bpool.tile([128, T], BF16, tag="o1")          # heads 2,3
        qpair = (qp0, qp1)

        nc.gpsimd.memset(vex[:, :, DH:DH + 1], 1.0)

        # ---------------- x load + transpose (PE) ----------------
        def load_x_tile(t):
            x_t = wpool.tile([128, DIN], BF16, tag="x_t", bufs=3, name=f"x_{t}")
            nc.gpsimd.dma_start(x_t, xap[128 * t:128 * (t + 1), :])
            for c in range(8):
                tp = ppool.tile([128, 128], BF16, tag="sm", bufs=2,
                                name=f"tp_{t}_{c}")
                nc.tensor.transpose(tp, x_t[:, 128 * c:128 * (c + 1)], idb)
                nc.vector.tensor_copy(xt[:, c, 128 * t:128 * (t + 1)], tp)

        # ---------------- per-512-token projections + RoPE ----------------
        def proj_nj(nj):
            sl = slice(512 * nj, 512 * (nj + 1))
            kvp = ppool.tile([128, 512], F32, tag="oac", bufs=1, name=f"kvp{nj}")
            qs0 = ppool.tile([128, 512], F32, tag="big2", bufs=2, name=f"qs0_{nj}")
            qs1 = ppool.tile([128, 512], F32, tag="big2", bufs=2, name=f"qs1_{nj}")
            for c in range(8):
                st, sp = (c == 0), (c == 7)
                nc.tensor.matmul(kvp, wkv[:, c, :], xt[:, c, sl], start=st, stop=sp)
                nc.tensor.matmul(qs0, wq[:, c, 0:128], xt[:, c, sl], start=st, stop=sp)
                nc.tensor.matmul(qs1, wq[:, c, 128:256], xt[:, c, sl], start=st, stop=sp)
            # RoPE q: q'[p] = q[p]*cos[p] + q[p^32]*s2[p]   (s2 sign-folded)
            # The rotated read hits the PSUM operand; both SBUF APs stay
            # base-aligned (walrus requires equal bases for two-SBUF inputs).
            for j, qs in enumerate((qs0, qs1)):
                m1 = wpool.tile([128, 512], F32, tag="m1", bufs=2, name=f"m1_{nj}_{j}")
                m2 = wpool.tile([128, 512], F32, tag="m2", bufs=2, name=f"m2_{nj}_{j}")
                nc.vector.tensor_mul(m1, qs, crep[:, sl])
                for b in range(4):
                    a0, a1 = 32 * b, 32 * (b + 1)
                    r0, r1 = 32 * (b ^ 1), 32 * ((b ^ 1) + 1)
                    nc.vector.tensor_mul(m2[a0:a1, :], qs[r0:r1, :],
                                         srep[a0:a1, sl])
                nc.vector.tensor_add(qpair[j][:, sl], m1, m2)
            # RoPE k (rows 0:64 of kvp), v copy (rows 64:128)
            km1 = wpool.tile([64, 512], F32, tag="m1", bufs=2, name=f"km1_{nj}")
            km2 = wpool.tile([64, 512], F32, tag="m2", bufs=2, name=f"km2_{nj}")
            nc.vector.tensor_mul(km1, kvp[0:64, :], crep[0:64, sl])
            nc.vector.tensor_mul(km2[0:32, :], kvp[32:64, :], srep[0:32, sl])
            nc.vector.tensor_mul(km2[32:64, :], kvp[0:32, :], srep[32:64, sl])
            nc.vector.tensor_add(k2[0:64, sl], km1, km2)
            nc.vector.tensor_copy(vst[:, sl], kvp[64:128, :])

        for nj in range(4):
            for t in range(4 * nj, 4 * nj + 4):
                load_x_tile(t)
            proj_nj(nj)

        # duplicate k rows so heads at partition-base 64 have aligned weights
        nc.vector.tensor_copy(k2[64:128, :], k2[0:64, :])

        # token-major V tiles (with ones column) via PE transpose
        for tt in range(16):
            vp = ppool.tile([128, 64], BF16, tag="sm", bufs=2, name=f"vp{tt}")
            nc.tensor.transpose(vp, vst[:, 128 * tt:128 * (tt + 1)],
                                idb[0:64, 0:64])
            nc.vector.tensor_copy(vex[:, tt, 0:DH], vp)

        # ---------------- attention + out-projection ----------------
        for J in range(2):
            for h in range(HPG):
                pj, po = h // 2, 64 * (h % 2)
                q_t = qpair[pj]
                oacc = ppool.tile([DH + 1, 1024], F32, tag="oac", bufs=1,
                                  name=f"oacc_{J}_{h}")
                n_i = 8 * (J + 1)
                for i in range(n_i):
                    isl = slice(128 * i, 128 * (i + 1))
                    sps = ppool.tile([128, 1024], F32, tag="big2", bufs=2,
                                     name=f"sps_{J}_{h}_{i}")
                    live = []
                    for m in range(2):
                        d = i - 8 * J - 4 * m
                        if d > 3:
                            continue  # half entirely above the diagonal
                        live.append((m, d))
                        tqsl = slice(1024 * J + 512 * m, 1024 * J + 512 * (m + 1))
                        osl = slice(512 * m, 512 * (m + 1))
                        nc.tensor.matmul(sps[:, osl], k2[po:po + 64, isl],
                                         q_t[po:po + 64, tqsl],
                                         start=True, stop=True)
                    m0 = live[0][0]
                    ex = wpool.tile([128, 1024], BF16, tag="ex", bufs=3,
                                    name=f"ex_{J}_{h}_{i}")
                    nc.scalar.activation(ex[:, 512 * m0:1024],
                                         sps[:, 512 * m0:1024], EXP, scale=SCALE)
                    for m, d in live:
                        if 0 <= d <= 3:
                            osl = slice(512 * m, 512 * (m + 1))
                            nc.gpsimd.tensor_mul(ex[:, osl], ex[:, osl],
                                                 masks[:, d, :])
                    for m, d in live:
                        osl = slice(512 * m, 512 * (m + 1))
                        nc.tensor.matmul(oacc[:, osl], vex[:, i, :], ex[:, osl],
                                         start=(i == 0),
                                         stop=(i == 8 * J + 4 * m + 3))
                # softmax denominators: row DH of oacc
                rec = wpool.tile([1, 1024], F32, tag="rec", bufs=2,
                                 name=f"rec_{J}_{h}")
                nc.vector.reciprocal(rec, oacc[DH:DH + 1, :])
                rbc = wpool.tile([64, 1024], F32, tag="rbc", bufs=2,
                                 name=f"rbc_{J}_{h}")
                nc.gpsimd.partition_broadcast(rbc, rec)
                otile = o0 if h < 2 else o1
                nc.vector.tensor_mul(otile[po:po + 64, 1024 * J:1024 * (J + 1)],
                                     oacc[0:DH, :], rbc)
            # out-projection for this 1024-token chunk
            for tq in range(8):
                tqc = 8 * J + tq
                csl = slice(128 * tqc, 128 * (tqc + 1))
                for n in range(2):
                    nsl = slice(512 * n, 512 * (n + 1))
                    ops = ppool.tile([128, 512], F32, tag="sm", bufs=2,
                                     name=f"ops_{tqc}_{n}")
                    nc.tensor.matmul(ops, o0[:, csl], wo[:, 0, nsl],
                                     start=True, stop=False)
                    nc.tensor.matmul(ops, o1[:, csl], wo[:, 1, nsl],
                                     start=False, stop=True)
                    oc = wpool.tile([128, 512], F32, tag="oc", bufs=3,
                                    name=f"oc_{tqc}_{n}")
                    nc.vector.tensor_copy(oc, ops)
                    nc.sync.dma_start(oap[csl, nsl], oc)


def _host_inputs(x, Wq, Wk, Wv, Wo, cos, sin):
    """Build the 8 per-core input dicts."""
    bf = ml_dtypes.bfloat16
    cos32 = np.ascontiguousarray(cos[:, :32].T)            # [32, T]
    sin32 = np.ascontiguousarray(sin[:, :32].T)
    crep = np.tile(cos32, (4, 1)).astype(bf)               # [128, T]
    # destination-indexed rotate sign: q'[p] = q[p]*c + q[p^32]*s2[p]
    # p in first half of a head (A rows): -sin; second half (B rows): +sin
    sgn = np.tile(sin32, (4, 1)).astype(np.float32)
    for blk in range(4):
        if blk % 2 == 0:                                   # rows 0..31 mod 64
            sgn[32 * blk:32 * (blk + 1)] *= -1.0
    srep = sgn.astype(bf)
    masks = np.zeros((128, 4, 512), dtype=np.float32)
    p = np.arange(128)[:, None]
    c = np.arange(512)[None, :]
    for d in range(4):
        masks[:, d, :] = (128 * d + p <= c).astype(np.float32)
    masks = masks.astype(bf)
    idb = np.eye(128, dtype=np.float32).astype(bf)

    in_maps = []
    for core in range(NCORES):
        b, g = divmod(core, 4)
        wkv = np.concatenate(
            [Wk[:, DH * g:DH * (g + 1)], Wv[:, DH * g:DH * (g + 1)]], axis=1)
        in_maps.append({
            "x": np.ascontiguousarray(x[b]).astype(np.float32),
            "wq": np.ascontiguousarray(Wq[:, QCOLS * g:QCOLS * (g + 1)]).astype(np.float32),
            "wkv": np.ascontiguousarray(wkv).astype(np.float32),
            "wo": np.ascontiguousarray(Wo[QCOLS * g:QCOLS * (g + 1), :]).astype(np.float32),
            "crep": crep,
            "srep": srep,
            "masks": masks,
            "idb": idb,
        })
    return in_maps


def _run(inputs, trace=False):
    from concourse.bass_utils import run_bass_kernel_spmd

    if "nc" not in _CACHE:
        _CACHE["nc"] = _build_nc()
    nc = _CACHE["nc"]
    in_maps = _host_inputs(**inputs)
    res = run_bass_kernel_spmd(nc, in_maps, core_ids=list(range(NCORES)),
                               trace=trace)
    parts = [r["out"] for r in res.results]
    out = np.stack([
        parts[0] + parts[1] + parts[2] + parts[3],
        parts[4] + parts[5] + parts[6] + parts[7],
    ]).astype(np.float32)
    return out, res


def kernel(x, Wq, Wk, Wv, Wo, cos, sin):
    out, _ = _run(dict(x=np.asarray(x), Wq=np.asarray(Wq), Wk=np.asarray(Wk),
                       Wv=np.asarray(Wv), Wo=np.asarray(Wo),
                       cos=np.asarray(cos), sin=np.asarray(sin)))
    return out


# revision 9
# speedup vs baseline: 1.2741x; 1.2741x over previous
"""GQA + RoPE + causal attention + out-proj, sharded over 8 NeuronCores.

Sharding: core = 4*b + g  (b = batch 0..1, g = KV group 0..3).
Each core computes q/k/v projections for its (batch, group), RoPE, causal
attention for its 4 query heads, and the partial out-projection through its
256 rows of Wo. The host sums the 4 group-partials per batch (the all-reduce
of the row-sharded out projection) and stacks batches.

On-chip layout: everything token-on-free ("transposed"): xT [din, tok] built
via PE transposes, qT/kT [dh, tok], scores computed as scoresT [tk, tq] so
that softmax denominators come for free from a ones-row appended to the
(token-major) V tiles, and attnT feeds both the AV matmul and the
out-projection without any attention-sized transposes.
Softmax skips max-subtraction: scores * T**-0.5 have |x| < 1 for this
problem's scale (weights ~ 0.02 * randn), so exp never overflows.
"""

import os
import sys

for _p in ("/opt/trn_rl_repo",):
    if _p not in sys.path and os.path.isdir(_p):
        sys.path.insert(0, _p)

import ml_dtypes
import numpy as np

import concourse.bacc as bacc
import concourse.mybir as mybir
import concourse.tile as tile

F32 = mybir.dt.float32
BF16 = mybir.dt.bfloat16
EXP = mybir.ActivationFunctionType.Exp

B, T, DIN, DOUT = 2, 2048, 1024, 1024
G, H = 4, 16
HPG = H // G          # 4 query heads per group
DH = DOUT // H        # 64
QCOLS = HPG * DH      # 256 q columns per group
SCALE = float(T) ** -0.5
NCORES = 8

_CACHE = {}


def _build_nc():
    nc = bacc.Bacc("TRN2", target_bir_lowering=False, debug=False,
                   num_devices=NCORES)

    x_d = nc.dram_tensor("x", [T, DIN], F32, kind="ExternalInput")
    wq_d = nc.dram_tensor("wq", [DIN, QCOLS], F32, kind="ExternalInput")
    wkv_d = nc.dram_tensor("wkv", [DIN, 2 * DH], F32, kind="ExternalInput")
    wo_d = nc.dram_tensor("wo", [QCOLS, DOUT], F32, kind="ExternalInput")
    crep_d = nc.dram_tensor("crep", [128, T], BF16, kind="ExternalInput")
    srep_d = nc.dram_tensor("srep", [128, T], BF16, kind="ExternalInput")
    idb_d = nc.dram_tensor("idb", [128, 128], BF16, kind="ExternalInput")
    out_d = nc.dram_tensor("out", [T, DOUT], F32, kind="ExternalOutput")

    with tile.TileContext(nc) as tc:
        _body(tc, nc, x_d, wq_d, wkv_d, wo_d, crep_d, srep_d, idb_d, out_d)
    nc.compile()
    return nc


def _body(tc, nc, x_d, wq_d, wkv_d, wo_d, crep_d, srep_d, idb_d, out_d):
    xap = x_d.ap()
    oap = out_d.ap()

    with (
        tc.tile_pool(name="cpool", bufs=1) as cpool,
        tc.tile_pool(name="bpool", bufs=1) as bpool,
        tc.tile_pool(name="wpool", bufs=1) as wpool,
        tc.tile_pool(name="ppool", bufs=1, space="PSUM") as ppool,
    ):
        # ---------------- constants / weights ----------------
        crep = cpool.tile([128, T], BF16, tag="crep")
        nc.sync.dma_start(crep, crep_d.ap())
        srep = cpool.tile([128, T], BF16, tag="srep")
        nc.sync.dma_start(srep, srep_d.ap())
        idb = cpool.tile([128, 128], BF16, tag="idb")
        nc.sync.dma_start(idb, idb_d.ap())

        wq = cpool.tile([128, 8, QCOLS], BF16, tag="wq")
        nc.gpsimd.dma_start(wq, wq_d.ap().rearrange("(c p) m -> p c m", p=128))
        wkv = cpool.tile([128, 8, 2 * DH], BF16, tag="wkv")
        nc.gpsimd.dma_start(wkv, wkv_d.ap().rearrange("(c p) m -> p c m", p=128))
        wo = cpool.tile([128, 2, DOUT], BF16, tag="wo")
        nc.gpsimd.dma_start(wo, wo_d.ap().rearrange("(r p) n -> p r n", p=128))

        # ---------------- persistent activations ----------------
        xt = bpool.tile([128, 8, T], BF16, tag="xt")       # xT, din chunk c
        qp0 = bpool.tile([128, T], BF16, tag="qp0")        # heads 0,1 (RoPEd)
        qp1 = bpool.tile([128, T], BF16, tag="qp1")        # heads 2,3
        k2 = bpool.tile([128, T], BF16, tag="k2")          # kT dup at base 0/64
        vst = bpool.tile([64, T], BF16, tag="vst")         # vT staging
        vex = bpool.tile([128, 16, DH + 1], BF16, tag="vex")  # [tok,65] + ones
        o0 = bpool.tile([128, T], BF16, tag="o0")          # o_gT heads 0,1
        o1 = bpool.tile([128, T], BF16, tag="o1")          # heads 2,3
        qpair = (qp0, qp1)

        nc.gpsimd.memset(vex[:, :, DH:DH + 1], 1.0)

        # ---------------- x load + transpose (PE) ----------------
        def load_x_tile(t):
            x_t = wpool.tile([128, DIN], BF16, tag="x_t", bufs=3, name=f"x_{t}")
            nc.gpsimd.dma_start(x_t, xap[128 * t:128 * (t + 1), :])
            for c4 in range(2):
                tp = ppool.tile([128, 512], BF16, tag="sm", bufs=2,
                                name=f"tp_{t}_{c4}")
                for k in range(4):
                    c = 4 * c4 + k
                    nc.tensor.transpose(tp[:, 128 * k:128 * (k + 1)],
                                        x_t[:, 128 * c:128 * (c + 1)], idb)
                # ACT is idle during the prologue; use it for psum->sbuf
                nc.scalar.copy(xt[:, 4 * c4:4 * c4 + 4, 128 * t:128 * (t + 1)],
                               tp)

        # ---------------- per-512-token projections + RoPE ----------------
        def proj_nj(nj):
            sl = slice(512 * nj, 512 * (nj + 1))
            kvp = ppool.tile([128, 512], F32, tag="oac", bufs=1, name=f"kvp{nj}")
            qs0 = ppool.tile([128, 512], F32, tag="big2", bufs=2, name=f"qs0_{nj}")
            qs1 = ppool.tile([128, 512], F32, tag="big2", bufs=2, name=f"qs1_{nj}")
            for c in range(8):
                st, sp = (c == 0), (c == 7)
                nc.tensor.matmul(kvp, wkv[:, c, :], xt[:, c, sl], start=st, stop=sp)
                nc.tensor.matmul(qs0, wq[:, c, 0:128], xt[:, c, sl], start=st, stop=sp)
                nc.tensor.matmul(qs1, wq[:, c, 128:256], xt[:, c, sl], start=st, stop=sp)
            # RoPE q: q'[p] = q[p]*cos[p] + q[p^32]*s2[p]   (s2 sign-folded)
            # The rotated read hits the PSUM operand; both SBUF APs stay
            # base-aligned (walrus requires equal bases for two-SBUF inputs).
            for j, qs in enumerate((qs0, qs1)):
                m1 = wpool.tile([128, 512], F32, tag="m1", bufs=2, name=f"m1_{nj}_{j}")
                m2 = wpool.tile([128, 512], F32, tag="m2", bufs=2, name=f"m2_{nj}_{j}")
                nc.vector.tensor_mul(m1, qs, crep[:, sl])
                for b in range(4):
                    a0, a1 = 32 * b, 32 * (b + 1)
                    r0, r1 = 32 * (b ^ 1), 32 * ((b ^ 1) + 1)
                    nc.vector.tensor_mul(m2[a0:a1, :], qs[r0:r1, :],
                                         srep[a0:a1, sl])
                nc.vector.tensor_add(qpair[j][:, sl], m1, m2)
            # RoPE k (rows 0:64 of kvp), v copy (rows 64:128)
            km1 = wpool.tile([64, 512], F32, tag="m1", bufs=2, name=f"km1_{nj}")
            km2 = wpool.tile([64, 512], F32, tag="m2", bufs=2, name=f"km2_{nj}")
            nc.vector.tensor_mul(km1, kvp[0:64, :], crep[0:64, sl])
            nc.vector.tensor_mul(km2[0:32, :], kvp[32:64, :], srep[0:32, sl])
            nc.vector.tensor_mul(km2[32:64, :], kvp[0:32, :], srep[32:64, sl])
            nc.vector.tensor_add(k2[0:64, sl], km1, km2)
            # duplicate k rows so heads at partition-base 64 have aligned
            # weights (per-chunk so attention J=0 can start after nj 0,1)
            nc.vector.tensor_copy(k2[64:128, sl], k2[0:64, sl])
            nc.vector.tensor_copy(vst[:, sl], kvp[64:128, :])

        for nj in range(4):
            for t in range(4 * nj, 4 * nj + 4):
                load_x_tile(t)
            proj_nj(nj)

        # token-major V tiles (with ones column) via PE transpose
        for tt in range(16):
            vp = ppool.tile([128, 64], BF16, tag="sm", bufs=2, name=f"vp{tt}")
            nc.tensor.transpose(vp, vst[:, 128 * tt:128 * (tt + 1)],
                                idb[0:64, 0:64])
            nc.vector.tensor_copy(vex[:, tt, 0:DH], vp)

        # ---------------- attention + out-projection ----------------
        for J in range(2):
            for h in range(HPG):
                pj, po = h // 2, 64 * (h % 2)
                q_t = qpair[pj]
                oacc = ppool.tile([DH + 1, 1024], F32, tag="oac", bufs=1,
                                  name=f"oacc_{J}_{h}")
                n_i = 8 * (J + 1)
                for i in range(n_i):
                    isl = slice(128 * i, 128 * (i + 1))
                    sps = ppool.tile([128, 1024], F32, tag="big2", bufs=2,
                                     name=f"sps_{J}_{h}_{i}")
                    live = []
                    for m in range(2):
                        d = i - 8 * J - 4 * m
                        if d > 3:
                            continue  # half entirely above the diagonal
                        live.append((m, d))
                        tqsl = slice(1024 * J + 512 * m, 1024 * J + 512 * (m + 1))
                        osl = slice(512 * m, 512 * (m + 1))
                        nc.tensor.matmul(sps[:, osl], k2[po:po + 64, isl],
                                         q_t[po:po + 64, tqsl],
                                         start=True, stop=True)
                    m0 = live[0][0]
                    ex = wpool.tile([128, 1024], BF16, tag="ex", bufs=4,
                                    name=f"ex_{J}_{h}_{i}")
                    nc.scalar.activation(ex[:, 512 * m0:1024],
                                         sps[:, 512 * m0:1024], EXP, scale=SCALE)
                    for m, d in live:
                        if 0 <= d <= 3:
                            # causal: keep ex[p,c] iff 128*d + p <= c
                            osl = slice(512 * m, 512 * (m + 1))
                            nc.gpsimd.affine_select(
                                ex[:, osl], ex[:, osl],
                                pattern=[[1, 512]],
                                compare_op=mybir.AluOpType.is_ge,
                                fill=0.0, base=-128 * d,
                                channel_multiplier=-1)
                    for m, d in live:
                        osl = slice(512 * m, 512 * (m + 1))
                        nc.tensor.matmul(oacc[:, osl], vex[:, i, :], ex[:, osl],
                                         start=(i == 0),
                                         stop=(i == 8 * J + 4 * m + 3))
                # softmax denominators: row DH of oacc. Broadcast the raw
                # denominator row first so the reciprocal runs on 64 lanes
                # (a [1, N] reciprocal is ~6.5us; [64, N] is ~1us).
                dnm = wpool.tile([1, 1024], F32, tag="rec", bufs=2,
                                 name=f"dnm_{J}_{h}")
                nc.vector.tensor_copy(dnm, oacc[DH:DH + 1, :])
                rbc = wpool.tile([64, 1024], F32, tag="rbc", bufs=2,
                                 name=f"rbc_{J}_{h}")
                nc.gpsimd.partition_broadcast(rbc, dnm)
                nc.vector.reciprocal(rbc, rbc)
                otile = o0 if h < 2 else o1
                nc.vector.tensor_mul(otile[po:po + 64, 1024 * J:1024 * (J + 1)],
                                     oacc[0:DH, :], rbc)
            # out-projection for this 1024-token chunk
            for tq in range(8):
                tqc = 8 * J + tq
                csl = slice(128 * tqc, 128 * (tqc + 1))
                for n in range(2):
                    nsl = slice(512 * n, 512 * (n + 1))
                    ops = ppool.tile([128, 512], F32, tag="sm", bufs=2,
                                     name=f"ops_{tqc}_{n}")
                    nc.tensor.matmul(ops, o0[:, csl], wo[:, 0, nsl],
                                     start=True, stop=False)
                    nc.tensor.matmul(ops, o1[:, csl], wo[:, 1, nsl],
                                     start=False, stop=True)
                    oc = wpool.tile([128, 512], F32, tag="oc", bufs=3,
                                    name=f"oc_{tqc}_{n}")
                    nc.vector.tensor_copy(oc, ops)
                    nc.sync.dma_start(oap[csl, nsl], oc)


def _host_inputs(x, Wq, Wk, Wv, Wo, cos, sin):
    """Build the 8 per-core input dicts."""
    bf = ml_dtypes.bfloat16
    cos32 = np.ascontiguousarray(cos[:, :32].T)            # [32, T]
    sin32 = np.ascontiguousarray(sin[:, :32].T)
    crep = np.tile(cos32, (4, 1)).astype(bf)               # [128, T]
    # destination-indexed rotate sign: q'[p] = q[p]*c + q[p^32]*s2[p]
    # p in first half of a head (A rows): -sin; second half (B rows): +sin
    sgn = np.tile(sin32, (4, 1)).astype(np.float32)
    for blk in range(4):
        if blk % 2 == 0:                                   # rows 0..31 mod 64
            sgn[32 * blk:32 * (blk + 1)] *= -1.0
    srep = sgn.astype(bf)
    idb = np.eye(128, dtype=np.float32).astype(bf)

    in_maps = []
    for core in range(NCORES):
        b, g = divmod(core, 4)
        wkv = np.concatenate(
            [Wk[:, DH * g:DH * (g + 1)], Wv[:, DH * g:DH * (g + 1)]], axis=1)
        in_maps.append({
            "x": np.ascontiguousarray(x[b]).astype(np.float32),
            "wq": np.ascontiguousarray(Wq[:, QCOLS * g:QCOLS * (g + 1)]).astype(np.float32),
            "wkv": np.ascontiguousarray(wkv).astype(np.float32),
            "wo": np.ascontiguousarray(Wo[QCOLS * g:QCOLS * (g + 1), :]).astype(np.float32),
            "crep": crep,
            "srep": srep,
            "idb": idb,
        })
    return in_maps


def _run(inputs, trace=False):
    from concourse.bass_utils import run_bass_kernel_spmd

    if "nc" not in _CACHE:
        _CACHE["nc"] = _build_nc()
    nc = _CACHE["nc"]
    in_maps = _host_inputs(**inputs)
    res = run_bass_kernel_spmd(nc, in_maps, core_ids=list(range(NCORES)),
                               trace=trace)
    parts = [r["out"] for r in res.results]
    out = np.stack([
        parts[0] + parts[1] + parts[2] + parts[3],
        parts[4] + parts[5] + parts[6] + parts[7],
    ]).astype(np.float32)
    return out, res


def kernel(x, Wq, Wk, Wv, Wo, cos, sin):
    out, _ = _run(dict(x=np.asarray(x), Wq=np.asarray(Wq), Wk=np.asarray(Wk),
                       Wv=np.asarray(Wv), Wo=np.asarray(Wo),
                       cos=np.asarray(cos), sin=np.asarray(sin)))
    return out


# revision 10
# speedup vs baseline: 1.5241x; 1.1962x over previous
"""GQA + RoPE + causal attention + out-proj, sharded over 8 NeuronCores.

Sharding: core = 4*b + g  (b = batch 0..1, g = KV group 0..3).
Each core computes q/k/v projections for its (batch, group), RoPE, causal
attention for its 4 query heads, and the partial out-projection through its
256 rows of Wo. The host sums the 4 group-partials per batch (the all-reduce
of the row-sharded out projection) and stacks batches.

On-chip layout: everything token-on-free ("transposed"): xT [din, tok] built
via PE transposes, qT/kT [dh, tok], scores computed as scoresT [tk, tq] so
that softmax denominators come for free from a ones-row appended to the
(token-major) V tiles, and attnT feeds both the AV matmul and the
out-projection without any attention-sized transposes.
Softmax skips max-subtraction: scores * T**-0.5 have |x| < 1 for this
problem's scale (weights ~ 0.02 * randn), so exp never overflows.
"""

import os
import sys

for _p in ("/opt/trn_rl_repo",):
    if _p not in sys.path and os.path.isdir(_p):
        sys.path.insert(0, _p)

import ml_dtypes
import numpy as np

import concourse.bacc as bacc
import concourse.mybir as mybir
import concourse.tile as tile

F32 = mybir.dt.float32
BF16 = mybir.dt.bfloat16
EXP = mybir.ActivationFunctionType.Exp

B, T, DIN, DOUT = 2, 2048, 1024, 1024
G, H = 4, 16
HPG = H // G          # 4 query heads per group
DH = DOUT // H        # 64
QCOLS = HPG * DH      # 256 q columns per group
SCALE = float(T) ** -0.5
NCORES = 8

_CACHE = {}


def _build_nc():
    nc = bacc.Bacc("TRN2", target_bir_lowering=False, debug=False,
                   num_devices=NCORES)

    x_d = nc.dram_tensor("x", [T, DIN], F32, kind="ExternalInput")
    wq_d = nc.dram_tensor("wq", [DIN, QCOLS], F32, kind="ExternalInput")
    wkv_d = nc.dram_tensor("wkv", [DIN, 2 * DH], F32, kind="ExternalInput")
    wo_d = nc.dram_tensor("wo", [QCOLS, DOUT], F32, kind="ExternalInput")
    crep_d = nc.dram_tensor("crep", [128, T], BF16, kind="ExternalInput")
    srep_d = nc.dram_tensor("srep", [128, T], BF16, kind="ExternalInput")
    idb_d = nc.dram_tensor("idb", [128, 128], BF16, kind="ExternalInput")
    out_d = nc.dram_tensor("out", [T, DOUT], F32, kind="ExternalOutput")

    with tile.TileContext(nc) as tc:
        _body(tc, nc, x_d, wq_d, wkv_d, wo_d, crep_d, srep_d, idb_d, out_d)
    nc.compile()
    return nc


def _body(tc, nc, x_d, wq_d, wkv_d, wo_d, crep_d, srep_d, idb_d, out_d):
    xap = x_d.ap()
    oap = out_d.ap()

    with (
        tc.tile_pool(name="cpool", bufs=1) as cpool,
        tc.tile_pool(name="bpool", bufs=1) as bpool,
        tc.tile_pool(name="wpool", bufs=1) as wpool,
        tc.tile_pool(name="ppool", bufs=1, space="PSUM") as ppool,
    ):
        # ---------------- constants / weights ----------------
        crep = cpool.tile([128, T], BF16, tag="crep")
        nc.sync.dma_start(crep, crep_d.ap())
        srep = cpool.tile([128, T], BF16, tag="srep")
        nc.sync.dma_start(srep, srep_d.ap())
        idb = cpool.tile([128, 128], BF16, tag="idb")
        nc.sync.dma_start(idb, idb_d.ap())

        wq = cpool.tile([128, 8, QCOLS], BF16, tag="wq")
        nc.gpsimd.dma_start(wq, wq_d.ap().rearrange("(c p) m -> p c m", p=128))
        wkv = cpool.tile([128, 8, 2 * DH], BF16, tag="wkv")
        nc.gpsimd.dma_start(wkv, wkv_d.ap().rearrange("(c p) m -> p c m", p=128))
        wo = cpool.tile([128, 2, DOUT], BF16, tag="wo")
        nc.gpsimd.dma_start(wo, wo_d.ap().rearrange("(r p) n -> p r n", p=128))

        # ---------------- persistent activations ----------------
        xt = bpool.tile([128, 8, T], BF16, tag="xt")       # xT, din chunk c
        qp0 = bpool.tile([128, T], BF16, tag="qp0")        # heads 0,1 (RoPEd)
        qp1 = bpool.tile([128, T], BF16, tag="qp1")        # heads 2,3
        k2 = bpool.tile([128, T], BF16, tag="k2")          # kT dup at base 0/64
        vst = bpool.tile([64, T], BF16, tag="vst")         # vT staging
        vex = bpool.tile([128, 16, DH + 1], BF16, tag="vex")  # [tok,65] + ones
        o0 = bpool.tile([128, T], BF16, tag="o0")          # o_gT heads 0,1
        o1 = bpool.tile([128, T], BF16, tag="o1")          # heads 2,3
        qpair = (qp0, qp1)

        nc.gpsimd.memset(vex[:, :, DH:DH + 1], 1.0)

        # ---------------- x load + transpose (PE) ----------------
        def load_x_tile(t):
            x_t = wpool.tile([128, DIN], BF16, tag="x_t", bufs=3, name=f"x_{t}")
            nc.gpsimd.dma_start(x_t, xap[128 * t:128 * (t + 1), :])
            for c4 in range(2):
                tp = ppool.tile([128, 512], BF16, tag="sm", bufs=2,
                                name=f"tp_{t}_{c4}")
                for k in range(4):
                    c = 4 * c4 + k
                    nc.tensor.transpose(tp[:, 128 * k:128 * (k + 1)],
                                        x_t[:, 128 * c:128 * (c + 1)], idb)
                # ACT is idle during the prologue; use it for psum->sbuf
                nc.scalar.copy(xt[:, 4 * c4:4 * c4 + 4, 128 * t:128 * (t + 1)],
                               tp)

        # ---------------- per-512-token projections + RoPE ----------------
        def proj_nj(nj):
            sl = slice(512 * nj, 512 * (nj + 1))
            kvp = ppool.tile([128, 512], F32, tag="oac", bufs=1, name=f"kvp{nj}")
            qs0 = ppool.tile([128, 512], F32, tag="big2", bufs=2, name=f"qs0_{nj}")
            qs1 = ppool.tile([128, 512], F32, tag="big2", bufs=2, name=f"qs1_{nj}")
            for c in range(8):
                st, sp = (c == 0), (c == 7)
                nc.tensor.matmul(kvp, wkv[:, c, :], xt[:, c, sl], start=st, stop=sp)
                nc.tensor.matmul(qs0, wq[:, c, 0:128], xt[:, c, sl], start=st, stop=sp)
                nc.tensor.matmul(qs1, wq[:, c, 128:256], xt[:, c, sl], start=st, stop=sp)
            # RoPE q: q'[p] = q[p]*cos[p] + q[p^32]*s2[p]   (s2 sign-folded)
            # The rotated read hits the PSUM operand; both SBUF APs stay
            # base-aligned (walrus requires equal bases for two-SBUF inputs).
            for j, qs in enumerate((qs0, qs1)):
                m1 = wpool.tile([128, 512], F32, tag="m1", bufs=2, name=f"m1_{nj}_{j}")
                m2 = wpool.tile([128, 512], F32, tag="m2", bufs=2, name=f"m2_{nj}_{j}")
                nc.vector.tensor_mul(m1, qs, crep[:, sl])
                for b in range(4):
                    a0, a1 = 32 * b, 32 * (b + 1)
                    r0, r1 = 32 * (b ^ 1), 32 * ((b ^ 1) + 1)
                    nc.vector.tensor_mul(m2[a0:a1, :], qs[r0:r1, :],
                                         srep[a0:a1, sl])
                nc.vector.tensor_add(qpair[j][:, sl], m1, m2)
            # RoPE k (rows 0:64 of kvp), v copy (rows 64:128)
            km1 = wpool.tile([64, 512], F32, tag="m1", bufs=2, name=f"km1_{nj}")
            km2 = wpool.tile([64, 512], F32, tag="m2", bufs=2, name=f"km2_{nj}")
            nc.vector.tensor_mul(km1, kvp[0:64, :], crep[0:64, sl])
            nc.vector.tensor_mul(km2[0:32, :], kvp[32:64, :], srep[0:32, sl])
            nc.vector.tensor_mul(km2[32:64, :], kvp[0:32, :], srep[32:64, sl])
            nc.vector.tensor_add(k2[0:64, sl], km1, km2)
            # duplicate k rows so heads at partition-base 64 have aligned
            # weights (per-chunk so attention J=0 can start after nj 0,1)
            nc.vector.tensor_copy(k2[64:128, sl], k2[0:64, sl])
            nc.vector.tensor_copy(vst[:, sl], kvp[64:128, :])

        for nj in range(4):
            for t in range(4 * nj, 4 * nj + 4):
                load_x_tile(t)
            proj_nj(nj)

        # token-major V tiles (with ones column) via PE transpose
        for tt in range(16):
            vp = ppool.tile([128, 64], BF16, tag="sm", bufs=2, name=f"vp{tt}")
            nc.tensor.transpose(vp, vst[:, 128 * tt:128 * (tt + 1)],
                                idb[0:64, 0:64])
            nc.vector.tensor_copy(vex[:, tt, 0:DH], vp)

        # ---------------- attention + out-projection ----------------
        for J in range(2):
            for h in range(HPG):
                pj, po = h // 2, 64 * (h % 2)
                q_t = qpair[pj]
                oacc = ppool.tile([DH + 1, 1024], F32, tag="oac", bufs=1,
                                  name=f"oacc_{J}_{h}")
                n_i = 8 * (J + 1)
                for i in range(n_i):
                    isl = slice(128 * i, 128 * (i + 1))
                    sps = ppool.tile([128, 1024], F32, tag="big2", bufs=2,
                                     name=f"sps_{J}_{h}_{i}")
                    live = []
                    for m in range(2):
                        d = i - 8 * J - 4 * m
                        if d > 3:
                            continue  # half entirely above the diagonal
                        live.append((m, d))
                        tqsl = slice(1024 * J + 512 * m, 1024 * J + 512 * (m + 1))
                        osl = slice(512 * m, 512 * (m + 1))
                        nc.tensor.matmul(sps[:, osl], k2[po:po + 64, isl],
                                         q_t[po:po + 64, tqsl],
                                         start=True, stop=True)
                    m0 = live[0][0]
                    ex = wpool.tile([128, 1024], BF16, tag="ex", bufs=4,
                                    name=f"ex_{J}_{h}_{i}")
                    nc.scalar.activation(ex[:, 512 * m0:1024],
                                         sps[:, 512 * m0:1024], EXP, scale=SCALE)
                    for m, d in live:
                        if 0 <= d <= 3:
                            # causal: keep ex[p,c] iff 128*d + p <= c
                            osl = slice(512 * m, 512 * (m + 1))
                            nc.gpsimd.affine_select(
                                ex[:, osl], ex[:, osl],
                                pattern=[[1, 512]],
                                compare_op=mybir.AluOpType.is_ge,
                                fill=0.0, base=-128 * d,
                                channel_multiplier=-1)
                    for m, d in live:
                        osl = slice(512 * m, 512 * (m + 1))
                        nc.tensor.matmul(oacc[:, osl], vex[:, i, :], ex[:, osl],
                                         start=(i == 0),
                                         stop=(i == 8 * J + 4 * m + 3))
                # Stage oacc to SBUF right away so the PSUM accumulator is
                # free for the next head (keeps the PE dense / HAM warm).
                stg = wpool.tile([DH + 1, 1024], F32, tag="stg", bufs=2,
                                 name=f"stg_{J}_{h}")
                nc.vector.tensor_copy(stg, oacc)
                # DVE reciprocal cost is ~6.4ns per FREE element regardless of
                # partition count, so reshape the 1024 denominators across 128
                # partitions via a tiny SBUF->SBUF DMA round trip.
                d128 = wpool.tile([128, 8], F32, tag="d128", bufs=2,
                                  name=f"d128_{J}_{h}")
                nc.sync.dma_start(d128, stg[DH:DH + 1, :])
                nc.vector.reciprocal(d128, d128)
                r1 = wpool.tile([1, 1024], F32, tag="rec", bufs=2,
                                name=f"r1_{J}_{h}")
                nc.sync.dma_start(r1, d128)
                rbc = wpool.tile([64, 1024], F32, tag="rbc", bufs=2,
                                 name=f"rbc_{J}_{h}")
                nc.gpsimd.partition_broadcast(rbc, r1)
                otile = o0 if h < 2 else o1
                nc.vector.tensor_mul(otile[po:po + 64, 1024 * J:1024 * (J + 1)],
                                     stg[0:DH, :], rbc)
            # out-projection for this 1024-token chunk
            for tq in range(8):
                tqc = 8 * J + tq
                csl = slice(128 * tqc, 128 * (tqc + 1))
                for n in range(2):
                    nsl = slice(512 * n, 512 * (n + 1))
                    ops = ppool.tile([128, 512], F32, tag="sm", bufs=2,
                                     name=f"ops_{tqc}_{n}")
                    nc.tensor.matmul(ops, o0[:, csl], wo[:, 0, nsl],
                                     start=True, stop=False)
                    nc.tensor.matmul(ops, o1[:, csl], wo[:, 1, nsl],
                                     start=False, stop=True)
                    oc = wpool.tile([128, 512], F32, tag="oc", bufs=3,
                                    name=f"oc_{tqc}_{n}")
                    nc.vector.tensor_copy(oc, ops)
                    nc.sync.dma_start(oap[csl, nsl], oc)


def _host_inputs(x, Wq, Wk, Wv, Wo, cos, sin):
    """Build the 8 per-core input dicts."""
    bf = ml_dtypes.bfloat16
    cos32 = np.ascontiguousarray(cos[:, :32].T)            # [32, T]
    sin32 = np.ascontiguousarray(sin[:, :32].T)
    crep = np.tile(cos32, (4, 1)).astype(bf)               # [128, T]
    # destination-indexed rotate sign: q'[p] = q[p]*c + q[p^32]*s2[p]
    # p in first half of a head (A rows): -sin; second half (B rows): +sin
    sgn = np.tile(sin32, (4, 1)).astype(np.float32)
    for blk in range(4):
        if blk % 2 == 0:                                   # rows 0..31 mod 64
            sgn[32 * blk:32 * (blk + 1)] *= -1.0
    srep = sgn.astype(bf)
    idb = np.eye(128, dtype=np.float32).astype(bf)

    in_maps = []
    for core in range(NCORES):
        b, g = divmod(core, 4)
        wkv = np.concatenate(
            [Wk[:, DH * g:DH * (g + 1)], Wv[:, DH * g:DH * (g + 1)]], axis=1)
        in_maps.append({
            "x": np.ascontiguousarray(x[b]).astype(np.float32),
            "wq": np.ascontiguousarray(Wq[:, QCOLS * g:QCOLS * (g + 1)]).astype(np.float32),
            "wkv": np.ascontiguousarray(wkv).astype(np.float32),
            "wo": np.ascontiguousarray(Wo[QCOLS * g:QCOLS * (g + 1), :]).astype(np.float32),
            "crep": crep,
            "srep": srep,
            "idb": idb,
        })
    return in_maps


def _run(inputs, trace=False):
    from concourse.bass_utils import run_bass_kernel_spmd

    if "nc" not in _CACHE:
        _CACHE["nc"] = _build_nc()
    nc = _CACHE["nc"]
    in_maps = _host_inputs(**inputs)
    res = run_bass_kernel_spmd(nc, in_maps, core_ids=list(range(NCORES)),
                               trace=trace)
    parts = [r["out"] for r in res.results]
    out = np.stack([
        parts[0] + parts[1] + parts[2] + parts[3],
        parts[4] + parts[5] + parts[6] + parts[7],
    ]).astype(np.float32)
    return out, res


def kernel(x, Wq, Wk, Wv, Wo, cos, sin):
    out, _ = _run(dict(x=np.asarray(x), Wq=np.asarray(Wq), Wk=np.asarray(Wk),
                       Wv=np.asarray(Wv), Wo=np.asarray(Wo),
                       cos=np.asarray(cos), sin=np.asarray(sin)))
    return out
